# revision 8
# baseline (speedup 1.0000x reference)
"""GCN layer kernel for Trainium2: out[b] = D^-1/2 (A[b]+I) D^-1/2 H[b] B.

Data-parallel, one graph per NeuronCore, no collectives.

Host-side refactoring (all O(N^2) data prep; the device is a pure
single-pass streaming matmul):
    d    = 1/sqrt(1 + rowsum(A))           (host)
    ATs  = (D A D)^T  in bf16              (host; folds BOTH normalizations)
    h2t  = (D^2 H)^T  in bf16              (host; carries the +I self-loop term)
so the device computes
    YT_b = sum_t X_t^T @ ATs_t[:,b]  +  B^T @ h2t[:,b],   X_t = (H B) slab t
with one pass over ATs pipelined directly behind the DMA stream. bf16 halves
HBM traffic (8.4MB vs 16.8MB); rounding errors average out over the 2048-term
contraction (~4e-3 max rel vs the 2e-2 gate).

Layout/stream choices (from trace analysis):
 - ATs pre-slabbed on host to [128, 16*2048] so each chunk DMA is one
   contiguous 4-16KB run per partition (~425 GB/s sustained).
 - Everything runs on the Sync HWDGE queue; the Scalar queue measured
   ~144 GB/s vs Sync's ~425 on this platform.
 - Chunk sizes taper at BOTH ends [1,1,2,4,4,2,1,1]: early start for the
   PE pipeline, and sub-us tail so the PE isn't stuck waiting ~5us for a
   final 2MB chunk semaphore.
 - h2t ships second-to-last; the self-loop term is the accumulation's
   stop-pass, so it adds no head latency.
 - YT lives as 4 independent one-bank PSUM tiles so each 512-column
   block's evacuation depends only on its own stop matmul.
Output leaves as bf16 [O, N]; host upcasts + transposes.
"""
import sys

sys.path.insert(0, "/opt/trn_rl_repo")

import numpy as np
import ml_dtypes

BF16 = ml_dtypes.bfloat16
B_, N_, F_, O_ = 8, 2048, 128, 128
NT = N_ // 128  # 16 slabs
CHUNKS = [4, 4, 4, 2]  # slabs 0-13; slabs 14-15 stream as column chunks
N_CORES = 8

_CACHE = {}
LAST_RESULTS = None


def _build_program():
    import concourse.bacc as bacc
    import concourse.tile as tile
    import concourse.mybir as mybir

    f32 = mybir.dt.float32
    bf16 = mybir.dt.bfloat16

    nc = bacc.Bacc(None, target_bir_lowering=False)
    # packed: [p, t*N_+i] = ATs[t*128+p, i]
    ATS = nc.dram_tensor("ats", [128, NT * N_], bf16, kind="ExternalInput")
    # bw | ht
    HH = nc.dram_tensor("hh", [F_, 128 + N_], bf16, kind="ExternalInput")
    H2T = nc.dram_tensor("h2t", [F_, N_], bf16, kind="ExternalInput")
    OT = nc.dram_tensor("ot", [O_, N_], bf16, kind="ExternalOutput")

    chunk_start = []
    s0 = 0
    for csz in CHUNKS:
        chunk_start.append(s0)
        s0 += csz

    with tile.TileContext(nc) as tc:
        with (
            tc.tile_pool(name="const", bufs=1) as cst,
            tc.tile_pool(name="achunks", bufs=1) as ach,
            tc.tile_pool(name="xpool", bufs=1) as xpl,
            tc.tile_pool(name="outp", bufs=4) as outp,
            tc.tile_pool(name="psbig", bufs=1, space="PSUM") as psb,
            tc.tile_pool(name="pssmall", bufs=2, space="PSUM") as pss,
        ):
            hh_sb = cst.tile([128, 128 + N_], bf16, tag="hh")
            nc.sync.dma_start(out=hh_sb, in_=HH[:, :])
            bw = hh_sb[:, 0:128]
            ht = hh_sb[:, 128 : 128 + N_]
            h2t_sb = cst.tile([128, N_], bf16, tag="h2t")

            # ATs chunks (slabs 0-13) + h2t on the Sync HWDGE ring in stream
            # order, then slabs 14-15 as 8 interleaved 128KB column chunks so
            # the PE/evac/output tail pipelines per 512-col block instead of
            # stalling ~2.5us on one big final chunk's write-receipt.
            at_slab = [None] * NT
            for ci, csz in enumerate(CHUNKS):
                st = chunk_start[ci]
                t = ach.tile([128, csz * N_], bf16, tag=f"at{ci}")
                nc.sync.dma_start(out=t, in_=ATS[:, st * N_ : (st + csz) * N_])
                for sl in range(csz):
                    at_slab[st + sl] = t[:, sl * N_ : (sl + 1) * N_]
            nc.sync.dma_start(out=h2t_sb, in_=H2T[:, :])
            tail = {}
            for b in range(4):
                for s in (14, 15):
                    tt = ach.tile([128, 512], bf16, tag=f"at_s{s}b{b}")
                    nc.sync.dma_start(
                        out=tt, in_=ATS[:, s * N_ + b * 512 : s * N_ + (b + 1) * 512]
                    )
                    tail[(s, b)] = tt

            # X_t = (H @ B) slab t, evacuated to SBUF as bf16 stationaries
            xs = []
            for t in range(NT):
                x_t = xpl.tile([128, O_], bf16, tag=f"x{t}")
                xs.append(x_t)
            for t in range(NT):
                p_ps = pss.tile([128, O_], f32, tag="pp")
                nc.tensor.matmul(
                    p_ps, ht[:, t * 128 : (t + 1) * 128], bw, start=True, stop=True
                )
                nc.vector.tensor_copy(xs[t], p_ps)

            # 4 independent one-bank accumulators for YT's 512-col blocks
            yt = []
            for b in range(4):
                yt_b = psb.tile([128, 512], f32, tag=f"yt{b}")
                yt.append(yt_b)

            # main accumulation over slabs 0-13, one pass behind the stream
            for t in range(NT - 2):
                for b in range(4):
                    nc.tensor.matmul(
                        yt[b],
                        xs[t],
                        at_slab[t][:, b * 512 : (b + 1) * 512],
                        start=(t == 0),
                        stop=False,
                    )
            # block-major tail: slabs 14-15 + self-loop stop-pass + evac +
            # output per block, pipelined against the tail column chunks
            for b in range(4):
                nc.tensor.matmul(yt[b], xs[14], tail[(14, b)], start=False, stop=False)
                nc.tensor.matmul(yt[b], xs[15], tail[(15, b)], start=False, stop=False)
                nc.tensor.matmul(
                    yt[b],
                    bw,
                    h2t_sb[:, b * 512 : (b + 1) * 512],
                    start=False,
                    stop=True,
                )
                ost = outp.tile([128, 512], bf16, tag="ost")
                nc.vector.tensor_copy(ost, yt[b])
                nc.sync.dma_start(out=OT[:, b * 512 : (b + 1) * 512], in_=ost)

    nc.compile()
    return nc


def _get_program():
    if "nc" not in _CACHE:
        _CACHE["nc"] = _build_program()
    return _CACHE["nc"]


def kernel(H, A, B):
    global LAST_RESULTS
    from concourse.bass_utils import run_bass_kernel_spmd

    nc = _get_program()

    H32 = np.asarray(H, dtype=np.float32)
    A32 = np.asarray(A, dtype=np.float32)
    B16 = np.asarray(B, dtype=np.float32).astype(BF16)

    in_maps = []
    for b in range(B_):
        Ab = A32[b]
        dvec = (1.0 / np.sqrt(1.0 + Ab.sum(axis=1, dtype=np.float64))).astype(
            np.float32
        )
        ATs = (Ab * dvec[:, None] * dvec[None, :]).T  # [j, i] fp32
        ats_packed = (
            np.ascontiguousarray(ATs.reshape(NT, 128, N_).transpose(1, 0, 2))
            .reshape(128, NT * N_)
            .astype(BF16)
        )
        Hb = H32[b]
        hh = np.empty((F_, 128 + N_), dtype=BF16)
        hh[:, 0:128] = B16
        hh[:, 128:] = Hb.T.astype(BF16)
        h2t = np.ascontiguousarray((Hb * (dvec * dvec)[:, None]).T).astype(BF16)
        in_maps.append({"ats": ats_packed, "hh": hh, "h2t": h2t})

    res = run_bass_kernel_spmd(nc, in_maps, list(range(N_CORES)))
    LAST_RESULTS = res

    out = np.empty((B_, N_, O_), dtype=np.float32)
    for b in range(B_):
        out[b] = res.results[b]["ot"].astype(np.float32).T
    return out


# revision 9
# speedup vs baseline: 1.0555x; 1.0555x over previous
"""GCN layer kernel for Trainium2: out[b] = D^-1/2 (A[b]+I) D^-1/2 H[b] B.

Data-parallel, one graph per NeuronCore, no collectives.

Host-side refactoring (all O(N^2) data prep; the device is a pure
single-pass streaming matmul):
    d    = 1/sqrt(1 + rowsum(A))           (host)
    ATs  = (D A D)^T  in bf16              (host; folds BOTH normalizations)
    h2t  = (D^2 H)^T  in bf16              (host; carries the +I self-loop term)
so the device computes
    YT_b = sum_t X_t^T @ ATs_t[:,b]  +  B^T @ h2t[:,b],   X_t = (H B) slab t
with one pass over ATs pipelined directly behind the DMA stream. bf16 halves
HBM traffic (8.4MB vs 16.8MB); rounding errors average out over the 2048-term
contraction (~4e-3 max rel vs the 2e-2 gate).

Layout/stream choices (from trace analysis):
 - ATs pre-slabbed on host to [128, 16*2048] so each chunk DMA is one
   contiguous 4-16KB run per partition (~425 GB/s sustained).
 - Everything runs on the Sync HWDGE queue; the Scalar queue measured
   ~144 GB/s vs Sync's ~425 on this platform.
 - Chunk sizes taper at BOTH ends [1,1,2,4,4,2,1,1]: early start for the
   PE pipeline, and sub-us tail so the PE isn't stuck waiting ~5us for a
   final 2MB chunk semaphore.
 - h2t ships second-to-last; the self-loop term is the accumulation's
   stop-pass, so it adds no head latency.
 - YT lives as 4 independent one-bank PSUM tiles so each 512-column
   block's evacuation depends only on its own stop matmul.
Output leaves as bf16 [O, N]; host upcasts + transposes.
"""
import sys

sys.path.insert(0, "/opt/trn_rl_repo")

import numpy as np
import ml_dtypes

BF16 = ml_dtypes.bfloat16
B_, N_, F_, O_ = 8, 2048, 128, 128
NT = N_ // 128  # 16 slabs
CHUNKS = [4, 4, 4, 2]  # slabs 0-13; slabs 14-15 stream as column chunks
N_CORES = 8

_CACHE = {}
LAST_RESULTS = None


def _build_program():
    import concourse.bacc as bacc
    import concourse.tile as tile
    import concourse.mybir as mybir

    f32 = mybir.dt.float32
    bf16 = mybir.dt.bfloat16

    nc = bacc.Bacc(None, target_bir_lowering=False)
    # packed: [p, t*N_+i] = ATs[t*128+p, i]
    ATS = nc.dram_tensor("ats", [128, NT * N_], bf16, kind="ExternalInput")
    # bw | ht
    HH = nc.dram_tensor("hh", [F_, 128 + N_], bf16, kind="ExternalInput")
    H2T = nc.dram_tensor("h2t", [F_, N_], bf16, kind="ExternalInput")
    OT = nc.dram_tensor("ot", [O_, N_], bf16, kind="ExternalOutput")

    chunk_start = []
    s0 = 0
    for csz in CHUNKS:
        chunk_start.append(s0)
        s0 += csz

    with tile.TileContext(nc) as tc:
        with (
            tc.tile_pool(name="const", bufs=1) as cst,
            tc.tile_pool(name="achunks", bufs=1) as ach,
            tc.tile_pool(name="xpool", bufs=1) as xpl,
            tc.tile_pool(name="outp", bufs=4) as outp,
            tc.tile_pool(name="psbig", bufs=1, space="PSUM") as psb,
            tc.tile_pool(name="pssmall", bufs=2, space="PSUM") as pss,
        ):
            hh_sb = cst.tile([128, 128 + N_], bf16, tag="hh")
            nc.sync.dma_start(out=hh_sb, in_=HH[:, :])
            bw = hh_sb[:, 0:128]
            ht = hh_sb[:, 128 : 128 + N_]
            h2t_sb = cst.tile([128, N_], bf16, tag="h2t")

            # ATs chunks (slabs 0-13) + h2t on the Sync HWDGE ring in stream
            # order, then slabs 14-15 as 8 interleaved 128KB column chunks so
            # the PE/evac/output tail pipelines per 512-col block instead of
            # stalling ~2.5us on one big final chunk's write-receipt.
            at_slab = [None] * NT
            for ci, csz in enumerate(CHUNKS):
                st = chunk_start[ci]
                t = ach.tile([128, csz * N_], bf16, tag=f"at{ci}")
                nc.sync.dma_start(out=t, in_=ATS[:, st * N_ : (st + csz) * N_])
                for sl in range(csz):
                    at_slab[st + sl] = t[:, sl * N_ : (sl + 1) * N_]
            nc.sync.dma_start(out=h2t_sb, in_=H2T[:, :])
            tail = {}
            for b in range(4):
                for s in (14, 15):
                    tt = ach.tile([128, 512], bf16, tag=f"at_s{s}b{b}")
                    nc.sync.dma_start(
                        out=tt, in_=ATS[:, s * N_ + b * 512 : s * N_ + (b + 1) * 512]
                    )
                    tail[(s, b)] = tt

            # X_t = (H @ B) slab t, evacuated to SBUF as bf16 stationaries
            xs = []
            for t in range(NT):
                x_t = xpl.tile([128, O_], bf16, tag=f"x{t}")
                xs.append(x_t)
            for t in range(NT):
                p_ps = pss.tile([128, O_], f32, tag="pp")
                nc.tensor.matmul(
                    p_ps, ht[:, t * 128 : (t + 1) * 128], bw, start=True, stop=True
                )
                nc.vector.tensor_copy(xs[t], p_ps)

            # 4 independent one-bank accumulators for YT's 512-col blocks
            yt = []
            for b in range(4):
                yt_b = psb.tile([128, 512], f32, tag=f"yt{b}")
                yt.append(yt_b)

            # main accumulation over slabs 0-13, one pass behind the stream
            for t in range(NT - 2):
                for b in range(4):
                    nc.tensor.matmul(
                        yt[b],
                        xs[t],
                        at_slab[t][:, b * 512 : (b + 1) * 512],
                        start=(t == 0),
                        stop=False,
                    )
            # block-major tail: slabs 14-15 + self-loop stop-pass + evac +
            # output per block, pipelined against the tail column chunks
            for b in range(4):
                nc.tensor.matmul(yt[b], xs[14], tail[(14, b)], start=False, stop=False)
                nc.tensor.matmul(yt[b], xs[15], tail[(15, b)], start=False, stop=False)
                nc.tensor.matmul(
                    yt[b],
                    bw,
                    h2t_sb[:, b * 512 : (b + 1) * 512],
                    start=False,
                    stop=True,
                )
                ost = outp.tile([128, 512], bf16, tag="ost")
                nc.vector.tensor_copy(ost, yt[b])
                # alternate HWDGE rings so output issues pair up in parallel
                eng = nc.sync if b % 2 == 0 else nc.scalar
                eng.dma_start(out=OT[:, b * 512 : (b + 1) * 512], in_=ost)

    nc.compile()
    return nc


def _get_program():
    if "nc" not in _CACHE:
        _CACHE["nc"] = _build_program()
    return _CACHE["nc"]


def kernel(H, A, B):
    global LAST_RESULTS
    from concourse.bass_utils import run_bass_kernel_spmd

    nc = _get_program()

    H32 = np.asarray(H, dtype=np.float32)
    A32 = np.asarray(A, dtype=np.float32)
    B16 = np.asarray(B, dtype=np.float32).astype(BF16)

    in_maps = []
    for b in range(B_):
        Ab = A32[b]
        dvec = (1.0 / np.sqrt(1.0 + Ab.sum(axis=1, dtype=np.float64))).astype(
            np.float32
        )
        ATs = (Ab * dvec[:, None] * dvec[None, :]).T  # [j, i] fp32
        ats_packed = (
            np.ascontiguousarray(ATs.reshape(NT, 128, N_).transpose(1, 0, 2))
            .reshape(128, NT * N_)
            .astype(BF16)
        )
        Hb = H32[b]
        hh = np.empty((F_, 128 + N_), dtype=BF16)
        hh[:, 0:128] = B16
        hh[:, 128:] = Hb.T.astype(BF16)
        h2t = np.ascontiguousarray((Hb * (dvec * dvec)[:, None]).T).astype(BF16)
        in_maps.append({"ats": ats_packed, "hh": hh, "h2t": h2t})

    res = run_bass_kernel_spmd(nc, in_maps, list(range(N_CORES)))
    LAST_RESULTS = res

    out = np.empty((B_, N_, O_), dtype=np.float32)
    for b in range(B_):
        out[b] = res.results[b]["ot"].astype(np.float32).T
    return out
